# revision 42
# baseline (speedup 1.0000x reference)
"""DequantingLinear Trainium2 kernel — transposed-codes + cast-DMA design.

y = x @ W^T + b where W = (w_q - 128) * w_scales (GGML Q8_0-style, block=32),
b = (b_q - 128) * b_scales.

Sharding: column-parallel over out_features across 8 cores (1536 rows of W
per core).  Design vs the first-generation kernel (~89 us):

1. The int32 codes carry one useful byte; the host repacks them to uint8
   (pure storage change, values identical), cutting the dominant HBM
   stream 4x: 18.9 MB -> 4.72 MB per core.
2. The host pre-TRANSPOSES the code matrix to [in, out] layout (layout
   only, like the x transpose), so the PE consumes dequantized tiles
   directly: no PE transposes, no PSUM evacuation traffic.
3. Block-to-partition permutation: a k-tile of 128 i-rows normally spans 4
   quant blocks, making the scale operand a cross-partition gather.  We
   instead permute which i lands on which (k-tile, partition) slot so each
   lane's scale is constant per tile and the scale operand is an ordinary
   [128, 1536] step-1 fp16 tile: 16 "L0" k-tiles (lane p -> block p mod 96)
   and 8 "L1" k-tiles (lane p -> block 32 + p mod 64), covering each
   (block, j) exactly once.  x is permuted identically on the host.
4. Dequant path (HW-measured costs drove every choice):
   - The uint8->fp16 conversion rides the DMA: GPSIMD SWDGE descriptors
     cast in the SDMA datapath (measured exact), so no compute engine
     spends cycles widening codes.
   - DVE then does ONE tensor_mul per tile at the 2x_1p packed rate
     (938 ns per [128,1536] tile; scalar_tensor_tensor has NO fast mode
     and measured 1.74 us, ACT casts 1.55 us — both avoided).
   - The missing -128 shift: matmul is linear in lhsT and all tiles of a
     scale layout share their rhs, so 3 matmuls per layout with
     lhsT = -128 * sum(x k-tiles) restore it.  The sum is reduced in fp32
     and split into hi+lo fp16 parts (two matmul sets) so the correction
     is exact to fp32 precision.
   (GPSIMD elementwise ops share SBUF ports with DVE — running them
   concurrently measured DVE TTs at 3.7us instead of 0.94 — so GPSIMD does
   only DMA triggering here.)
5. Bias: ACT casts (bq-128)->fp16 (ACT is otherwise idle), DVE multiplies
   by the host-REPLICATED per-block scales (plain 1-D APs; a (1,48,32)
   sub-dim AP measured ~3x slower), added via K=1 matmuls against a ones
   row.
6. DMA choreography: SWDGE (GPSIMD-issued, own completion-sem space)
   carries xt + bias bytes + the 6 cast-chunks; the SP HWDGE ring carries
   scales + y.  Partition-major DRAM layouts keep every transfer at 128
   large descriptors (a [3200,64] rearranged xt DMA measured 15.6 us of
   descriptor generation; partition-major is ~1 us).

Two TRN2 toolchain quirks are handled explicitly (see _strip_self_waits
and _patch_drain_split): several instruction structs encode at most ONE
semaphore wait (walrus "Too many sync wait commands"), and the kernel-tail
drain's global-clock waits are pre-spread across SP nops.  Producers are
arranged to carry exactly one wait (one-buffer-per-tile pools, two head
absorbers for the scale tile); a post-pass drops provably redundant waits.
Barrier semaphores are reset between rounds, so the post-pass only dedupes
waits on monotonic sems (engine clocks + DMA lanes) — deduping a barrier
wait deadlocks the kernel (found the hard way).
"""

import sys

import numpy as np

for _p in ("/opt/trn_rl_repo", "/root/.axon_site/_ro/trn_rl_repo"):
    if _p not in sys.path:
        sys.path.append(_p)

B = 64          # batch (x is [64, 1, 3072])
IN = 3072       # in_features
OUT = 12288     # out_features
BLOCK = 32      # quant block
NB = IN // BLOCK            # 96 blocks per row
NCORES = 8
OSH = OUT // NCORES         # 1536 out features per core
KT = IN // 128              # 24 contraction k-tiles
NL0 = 16                    # k-tiles using scale layout L0
NG = 3                      # o-groups of N=512 per core
NBC = OSH // BLOCK          # 48 bias blocks per core

KT_PER_CHUNK = 4
NCHUNK = KT // KT_PER_CHUNK

_CACHE: dict = {}


def _patch_drain_split():
    """The TRN2 ISA gives every instruction exactly ONE inline wait slot;
    Tile's kernel-tail drain asks for the whole global clock (~11 sems) on a
    single instruction, which walrus sometimes refuses ("Too many sync wait
    commands").  Pre-spread those waits across one SP nop per semaphore; the
    drain's own waits then elide via the SP engine clock."""
    from concourse import tile as tile_mod

    if getattr(tile_mod.TileContext, "_drain_split_patched", False):
        return
    from concourse.vector_clock import ScopedClock, VectorClock

    orig = tile_mod.TileContext._drain_and_barrier

    def patched(self, tick_clock, wait_clock):
        gvc = tick_clock.global_clock
        n = len(gvc)
        for p in range(n):
            t = gvc[p]
            if t <= 0:
                continue
            vc = VectorClock([0] * n)
            vc.require_at_least(p, t)
            nop = self.nc.sync.nop(hint="drain_wait_split", nofuse=True)
            wait_clock.add_sem_waits(nop.ins, ScopedClock({None: vc}))
        return orig(self, tick_clock, wait_clock)

    tile_mod.TileContext._drain_and_barrier = patched
    tile_mod.TileContext._drain_split_patched = True


def _build_nc():
    import concourse.bass as bass
    import concourse.mybir as mybir
    from concourse.tile import TileContext
    from contextlib import ExitStack

    _patch_drain_split()

    f32 = mybir.dt.float32
    i32 = mybir.dt.int32
    f16 = mybir.dt.float16
    u8 = mybir.dt.uint8
    Copy = mybir.ActivationFunctionType.Copy

    nc = bass.Bass()
    # Host-permuted/transposed uint8 codes, partition-major: row p holds the
    # 24 k-tiles' o-rows for slot (kt, p) back to back.
    wqt = nc.declare_dram_parameter("wqt", [128, KT * OSH], u8, isOutput=False)
    # Scale layouts L0 | L1, each [128, 1536] fp16.
    sc = nc.declare_dram_parameter("sc", [128, 2 * OSH], f16, isOutput=False)
    # Host-permuted x^T (fp16), partition-major [128, 25*64].
    xt = nc.declare_dram_parameter("xt", [128, (KT + 1) * B], f16, isOutput=False)
    # bias bytes: [bq int32 x 1536 | bs fp16 replicated x32 -> 1536 values]
    bb = nc.declare_dram_parameter("bb", [1, 4 * OSH + 2 * OSH], u8, isOutput=False)
    y = nc.declare_dram_parameter("y", [B, OSH], f32, isOutput=True)

    with TileContext(nc) as tc, ExitStack() as ctx:
        const = ctx.enter_context(tc.tile_pool(name="const", bufs=1))
        # One buffer per tile (no reuse): each producer carries exactly ONE
        # sem wait.  (In-place DVE multiply was tried: operand overlap
        # disables the 2x_1p packed mode, 2.2x slower.)
        wp_pool = ctx.enter_context(tc.tile_pool(name="wp", bufs=KT))
        ysb_pool = ctx.enter_context(tc.tile_pool(name="ysb", bufs=1))
        py_pool = ctx.enter_context(tc.tile_pool(name="py", bufs=1, space="PSUM"))
        scrap_pool = ctx.enter_context(tc.tile_pool(name="scrap", bufs=1, space="PSUM"))

        # --- input DMAs --------------------------------------------------
        # SWDGE (GPSIMD) ring: xt + bias bytes first, then the 6 code
        # chunks with in-flight u8->fp16 cast.
        xt_sb = const.tile([128, (KT + 1) * B], f16)
        nc.gpsimd.dma_start(xt_sb[:], xt[:, :])
        bb_sb = const.tile([1, 4 * OSH + 2 * OSH], u8)
        nc.gpsimd.dma_start(bb_sb[:], bb[:, :])
        bq_sb = bb_sb[0:1, 0 : 4 * OSH].bitcast(i32)
        bsx_sb = bb_sb[0:1, 4 * OSH : 6 * OSH].bitcast(f16)

        CW = KT_PER_CHUNK * OSH
        chunks = []
        for c in range(NCHUNK):
            t = const.tile([128, CW], f16, name=f"wq16_{c}")
            nc.gpsimd.dma_start(t[:], wqt[:, c * CW : (c + 1) * CW])
            chunks.append(t)

        def wq16_slice(kt):
            c, r = divmod(kt, KT_PER_CHUNK)
            return chunks[c][:, r * OSH : (r + 1) * OSH]

        # SP HWDGE ring: scales in, y out.
        sc_sb = const.tile([128, 2 * OSH], f16)
        nc.sync.dma_start(sc_sb[:], sc[:, :])

        def sc_slice(kt):
            s = 0 if kt < NL0 else 1
            return sc_sb[:, s * OSH : (s + 1) * OSH]

        # --- small prologue ----------------------------------------------
        scr_d = const.tile([1, 8], f32)
        ones1 = const.tile([1, B], f16)
        nc.vector.memset(ones1[:], 1.0)
        # Touch the scale halves once on DVE so every later DVE consumer's
        # sc wait is engine-order-covered (then stripped).
        nc.vector.tensor_copy(scr_d[0:1, 0:1], sc_sb[0:1, 0:1])
        nc.vector.tensor_copy(scr_d[0:1, 1:2], sc_sb[0:1, OSH : OSH + 1])
        nc.vector.tensor_copy(scr_d[0:1, 2:3], bsx_sb[0:1, 0:1])
        # bias (bq-128) cast on the otherwise-idle ACT engine
        bias_q16 = const.tile([1, OSH], f16)
        nc.scalar.activation(bias_q16[:], bq_sb, Copy, bias=-128.0)
        bias16 = const.tile([1, OSH], f16)

        # PE wait-absorber for the one-time xt DMA (matmul LW struct carries
        # at most one sync wait).
        scrap = scrap_pool.tile([1, 4], f32)
        nc.tensor.matmul(
            scrap[0:1, 0:1], xt_sb[:, 0:1], xt_sb[:, 0:1], start=True, stop=True
        )

        # --- -128 correction lhsT per scale layout -----------------------
        # Every tile's multiply used RAW codes (q*s); the missing
        # -128*sum(x)^T @ s term is restored by matmuls.  fp32 reduce, then
        # an exact hi+lo fp16 split (matmul rhs is fp16, so lhsT must be
        # 16-bit; one rounding would cost ~4e-4 of the result).
        xsum16 = {}

        def _emit_xsum():
            for sel, k0, n in ((0, 0, NL0), (1, NL0, KT - NL0)):
                acc = const.tile([128, B], f32, name=f"xsumf{sel}")
                view = xt_sb[:, k0 * B : (k0 + n) * B].rearrange(
                    "p (n b) -> p b n", n=n
                )
                nc.vector.tensor_reduce(
                    acc[:], view, mybir.AxisListType.X, mybir.AluOpType.add
                )
                nc.vector.tensor_scalar_mul(acc[:], acc[:], -128.0)
                hi = const.tile([128, B], f16, name=f"xsumhi{sel}")
                nc.vector.tensor_copy(hi[:], acc[:])
                res = const.tile([128, B], f32, name=f"xsumr{sel}")
                nc.vector.tensor_tensor(
                    res[:], acc[:], hi[:], mybir.AluOpType.subtract
                )
                lo = const.tile([128, B], f16, name=f"xsumlo{sel}")
                nc.vector.tensor_copy(lo[:], res[:])
                xsum16[sel] = (hi, lo)

        # --- main pipeline ------------------------------------------------
        y_sb = ysb_pool.tile([B, OSH], f32)
        py = [py_pool.tile([B, 512], f32, name=f"py{g}") for g in range(NG)]

        for kt in range(KT):
            wp = wp_pool.tile([128, OSH], f16)
            nc.vector.tensor_mul(wp[:], wq16_slice(kt), sc_slice(kt))
            for g in range(NG):
                nc.tensor.matmul(
                    py[g][:],
                    xt_sb[:, B * kt : B * (kt + 1)],
                    wp[:, 512 * g : 512 * (g + 1)],
                    start=kt == 0,
                    stop=False,
                )
            if kt == 1:
                _emit_xsum()
            if kt == 4:
                for sel, (hi, lo) in xsum16.items():
                    for part in (hi, lo):
                        for g in range(NG):
                            nc.tensor.matmul(
                                py[g][:],
                                part[:],
                                sc_sb[:, sel * OSH + 512 * g : sel * OSH + 512 * (g + 1)],
                                start=False,
                                stop=False,
                            )
            if kt == 18:
                # bias16 = (bq-128) * bsx, fp16 2x TT; feeds the closing mms
                nc.vector.tensor_mul(bias16[:], bias_q16[:], bsx_sb)

        # bias via K=1 matmuls against the ones row, closing accumulation
        for g in range(NG):
            nc.tensor.matmul(
                py[g][:],
                ones1[0:1, :],
                bias16[0:1, 512 * g : 512 * (g + 1)],
                start=False,
                stop=True,
            )
        for g in range(NG):
            nc.scalar.copy(y_sb[:, 512 * g : 512 * (g + 1)], py[g][:])

        nc.sync.dma_start(y[:, :], y_sb[:])

    _strip_self_waits(nc, mybir)
    return nc


_ENGINE_SEM_PREFIX = {
    "PE": "PE_",
    "DVE": "DVE_",
    "Activation": "Activation_",
    "SP": "SP_",
}


def _strip_self_waits(nc, mybir):
    """Several TRN2 ISA instruction structs encode at most ONE sync wait
    (walrus: "Too many sync wait commands").  Two classes of Tile-emitted
    waits are redundant and safe to drop from instructions carrying >=2:

    1. Self-engine waits: an engine completes its own instructions in order.
    2. Waits already observed (same value or higher) by an EARLIER
       instruction on the same in-order engine.

    Pool (GPSIMD) is special: the 8 Q7 cores do NOT complete in a single
    program order (so Pool_ self-sem waits are load-bearing and never
    dropped), but the Pool NX sequencer still DISPATCHES in order, and sem
    waits gate dispatch: a wait on an external sem already waited for by an
    earlier Pool instruction is dispatch-covered and droppable.

    Only monotonic sems (engine clocks, DMA lanes) may be deduped: barrier
    sems are reset by sem-subtract between rounds, so a repeated wait value
    there is NOT redundant (deduping one deadlocks the kernel).
    """
    fn = nc.m.functions[0]
    observed: dict = {}
    _MONO = ("DMAHW", "DMASW", "PE_", "DVE_", "Activation_", "SP_", "Pool_")

    def _dedupable(w):
        return w.ant_name.startswith(_MONO)

    for b in fn.blocks:
        for inst in b.instructions:
            si = inst.sync_info
            if si is None or not si.on_wait:
                continue
            eng = str(inst.engine)
            if eng.split(".")[-1] == "Pool":
                keep = [
                    w
                    for w in si.on_wait
                    if w.ant_name.startswith("Pool")
                    or not _dedupable(w)
                    or observed.get((eng, w.ant_name), 0) < w.wait_value
                ]
                for w in keep:
                    if _dedupable(w) and not w.ant_name.startswith("Pool"):
                        k = (eng, w.ant_name)
                        observed[k] = max(observed.get(k, 0), w.wait_value)
                if len(keep) != len(si.on_wait):
                    inst.sync_info = mybir.SyncInfo(
                        on_wait=keep, on_update=si.on_update
                    )
                continue
            if len(si.on_wait) < 2:
                for w in si.on_wait:
                    if _dedupable(w):
                        k = (eng, w.ant_name)
                        observed[k] = max(observed.get(k, 0), w.wait_value)
                continue
            keep = [
                w
                for w in si.on_wait
                if not _dedupable(w)
                or observed.get((eng, w.ant_name), 0) < w.wait_value
            ]
            pref = _ENGINE_SEM_PREFIX.get(str(inst.engine).split(".")[-1])
            if pref is not None:
                keep = [w for w in keep if not w.ant_name.startswith(pref)]
            if len(keep) >= 2 and type(inst).__name__ == "InstDMACopy":
                if any(
                    not w.ant_name.startswith(("DMAHW", "DMASW")) for w in keep
                ):
                    keep = [
                        w
                        for w in keep
                        if not w.ant_name.startswith(("DMAHW", "DMASW"))
                    ]
            for w in keep:
                if _dedupable(w):
                    k = (eng, w.ant_name)
                    observed[k] = max(observed.get(k, 0), w.wait_value)
            if len(keep) != len(si.on_wait):
                inst.sync_info = mybir.SyncInfo(
                    on_wait=keep, on_update=si.on_update
                )


def _get_nc():
    if "nc" not in _CACHE:
        _CACHE["nc"] = _build_nc()
    return _CACHE["nc"]


def _slot_permutation():
    """slot (kt, p) -> global i = 32*block + j.  16 L0 k-tiles map lane p to
    block p mod 96 (j = kt for p<96, 16+kt else); 8 L1 k-tiles map lane p to
    block 32 + p mod 64 (j = 16+g for p<64, 24+g else).  Bijective onto
    0..3071 (each (block, j) covered exactly once)."""
    i_slot = np.empty((KT, 128), dtype=np.int64)
    p = np.arange(128)
    for kt in range(NL0):
        b = np.where(p < 96, p, p - 96)
        j = np.where(p < 96, kt, 16 + kt)
        i_slot[kt] = 32 * b + j
    for g in range(KT - NL0):
        b = 32 + (p % 64)
        j = np.where(p < 64, 16 + g, 24 + g)
        i_slot[NL0 + g] = 32 * b + j
    return i_slot


def _make_in_maps(x, w_q, w_scales, b_q, b_scales):
    i_slot = _slot_permutation()
    flat = i_slot.reshape(-1)
    p = np.arange(128)
    r0_idx = np.where(p < 96, p, p - 96)
    r1_idx = 32 + (p % 64)

    x2 = np.ascontiguousarray(x.reshape(B, IN), dtype=np.float32)
    xtp = np.zeros((IN + 128, B), dtype=np.float16)               # [3200, 64]
    xtp[:IN] = x2[:, flat].T.astype(np.float16)
    xtp[IN] = 1.0
    # partition-major: [128, 25*64], row p = slot (kt, p) over all k-tiles
    xtp = np.ascontiguousarray(
        xtp.reshape(KT + 1, 128, B).transpose(1, 0, 2).reshape(128, (KT + 1) * B)
    )

    W8 = w_q.reshape(OUT, IN).astype(np.uint8)
    W8g = W8[:, flat]                                             # [OUT, 3072]
    ws_full = np.asarray(w_scales)                                # [12288, 96]
    bq_full = np.ascontiguousarray(b_q.reshape(OUT))
    bs_full = np.ascontiguousarray(b_scales)

    in_maps = []
    for c in range(NCORES):
        o0, o1 = c * OSH, (c + 1) * OSH
        wqt_c = np.ascontiguousarray(
            W8g[o0:o1].T.reshape(KT, 128, OSH).transpose(1, 0, 2).reshape(128, KT * OSH)
        )
        ws_c = ws_full[o0:o1].astype(np.float16)                  # [1536, 96]
        L0 = ws_c[:, r0_idx].T                                    # [128, 1536]
        L1 = ws_c[:, r1_idx].T
        sc_c = np.ascontiguousarray(np.concatenate([L0, L1], axis=1))
        bsx = np.repeat(
            bs_full[o0 // BLOCK : o1 // BLOCK].astype(np.float16), BLOCK
        )                                                         # [1536] f16
        bb_c = np.frombuffer(
            bq_full[o0:o1].astype("<i4").tobytes() + bsx.tobytes(),
            dtype=np.uint8,
        ).reshape(1, 6 * OSH)
        in_maps.append(
            {
                "wqt": wqt_c,
                "sc": sc_c,
                "xt": xtp,
                "bb": bb_c,
            }
        )
    return in_maps


def run_shards(x, w_q, w_scales, b_q, b_scales, trace=False):
    """Run the SPMD kernel; returns (y_full, BassKernelResults)."""
    from concourse.bass_utils import run_bass_kernel_spmd

    nc = _get_nc()
    in_maps = _make_in_maps(x, w_q, w_scales, b_q, b_scales)
    res = run_bass_kernel_spmd(
        nc, in_maps, core_ids=list(range(NCORES)), trace=trace
    )
    shards = [np.asarray(res.results[c]["y"]) for c in range(NCORES)]
    y = np.concatenate(shards, axis=1).reshape(B, 1, OUT)
    return y, res


def kernel(**inputs):
    y, _ = run_shards(
        inputs["x"],
        inputs["w_q"],
        inputs["w_scales"],
        inputs["b_q"],
        inputs["b_scales"],
        trace=False,
    )
    return y.astype(np.float32)
